# revision 30
# baseline (speedup 1.0000x reference)
"""AWQ int4 column-parallel linear for 8 Trainium2 NeuronCores.

y = x @ W^T, W[o,k] = (nib[o,k] - z[g,o]) * s[g,o], g = k // 128.

Sharding: out_features (11008) split into 8 contiguous shards of 1376
(column-parallel); x replicated; per-core outputs concatenated.

Per-core device kernel:
  - qweight shard arrives transposed+tiled (4, 128, 1376) int32
    [P, pp, o]; nibble j of pack (P, pp) is k = 1024*P + 8*pp + j.
  - dequant on DVE: (qw >> 4j) & 15 -> (nib - zb) * sb -> fp16 W tiles
    [128 k, 1376 o], one per contraction tile ct = 8P + j. zb/sb are
    replicated across the 16 partitions of each group via broadcast DMA.
  - x arrives pre-transposed/permuted (32, 128, 4096) fp32 [ct, pp, t];
    each token tile is DMA'd with an in-flight fp32->fp16 cast (SWDGE).
  - TensorE: 32 t-tiles x 3 o-tiles(512/512/352) x 32 k-tiles of fp16
    matmuls accumulated in PSUM fp32; ScalarE copies PSUM->SBUF; HWDGE
    stores to y.

fp16 keeps 10 mantissa bits on both operands (measured ~1.5e-4 rel err
class, vs 2.3e-3 for bf16) at bf16 throughput, and halves SBUF so the
full W shard stays resident and x streams exactly once.
"""

import numpy as np

TOKENS = 4096
IN_F = 4096
OUT_F = 11008
GROUP = 128
N_CORES = 8
O_SH = OUT_F // N_CORES      # 1376
PACKS = IN_F // 8            # 512
NP_T = PACKS // 128          # 4 pack-tiles
CT = NP_T * 8                # 32 contraction tiles
G = IN_F // GROUP            # 32 quant groups
T_TILE = 128
N_T = TOKENS // T_TILE       # 32 token tiles


def _o_tiles(o_sh):
    """Split the o shard into PSUM-sized (<=512) column tiles."""
    tiles = []
    off = 0
    while off < o_sh:
        n = min(512, o_sh - off)
        tiles.append((off, n))
        off += n
    return tiles


def _build(n_t=N_T, o_sh=O_SH):
    import concourse.bacc as bacc
    import concourse.mybir as mybir
    import concourse.tile as tile

    dt = mybir.dt
    nc = bacc.Bacc("TRN2", target_bir_lowering=False, debug=False)

    xt_d = nc.dram_tensor("xt", [n_t, NP_T, 128, 8, T_TILE], dt.float32,
                          kind="ExternalInput").ap()
    qw_d = nc.dram_tensor("qw", [NP_T, 128, o_sh], dt.int32,
                          kind="ExternalInput").ap()
    sc_d = nc.dram_tensor("sc", [G, o_sh], dt.float16,
                          kind="ExternalInput").ap()
    zr_d = nc.dram_tensor("zr", [G, o_sh], dt.float16,
                          kind="ExternalInput").ap()
    y_d = nc.dram_tensor("y", [n_t * T_TILE, o_sh], dt.float32,
                         kind="ExternalOutput").ap()

    with tile.TileContext(nc) as tc:
        with (
            tc.tile_pool(name="qpool", bufs=2) as qpool,
            tc.tile_pool(name="wk", bufs=2) as wk,
            tc.tile_pool(name="wpool", bufs=1) as wpool,
            tc.tile_pool(name="xstage", bufs=4) as xstage,
            tc.tile_pool(name="xpool", bufs=4) as xpool,
            tc.tile_pool(name="opool", bufs=3) as opool,
            tc.tile_pool(name="psum", bufs=8, space="PSUM") as pspool,
        ):
            # x tile production: SWDGE casting DMA (fp32 DRAM -> fp16
            # SBUF), split into 4 chunk-DMAs per t-tile so the first
            # matmul gates on a 512KB chunk, not the whole 2MB (the
            # cast path runs well below line rate).
            def emit_x(tt):
                tiles = []
                for P in range(NP_T):
                    xr = xpool.tile([128, 8, T_TILE], dt.float16,
                                    tag=f"xr{P}", name=f"xr_{tt}_{P}")
                    nc.gpsimd.dma_start(out=xr[:], in_=xt_d[tt, P])
                    tiles.append(xr)
                return tiles
            # x tile production, startup: fp32 chunk DMA on the ACT
            # HWDGE ring (separate from the sync ring carrying qw/sc/
            # zr) + ACT cast. Chunks are emitted in waves matched to
            # the interleaved matmul block's progress through ct so
            # the DMA fabric isn't flooded ahead of the qw/zb/sb
            # transfers that gate dequant.
            def emit_x_chunk(tt, P):
                st = xstage.tile([128, 8, T_TILE], dt.float32,
                                 tag="xs", name=f"xs_{tt}_{P}")
                nc.scalar.dma_start(out=st[:], in_=xt_d[tt, P])
                xr = xpool.tile([128, 8, T_TILE], dt.float16,
                                tag=f"xr{P}", name=f"xr_{tt}_{P}")
                nc.scalar.copy(xr[:], st[:])
                return xr

            FAST_X = min(3, n_t)
            xr_pre = {tt: [] for tt in range(FAST_X)}
            # P0's scale/zero broadcasts go FIRST on the ACT ring (FIFO
            # per ring — anything ahead of them delays the first W tile)
            sbb0 = qpool.tile([128, o_sh], dt.float16, tag="sb")
            nc.scalar.dma_start(
                out=sbb0[:],
                in_=sc_d[0:8].unsqueeze(1).broadcast_to([8, 16, o_sh]))
            zbb0 = qpool.tile([128, o_sh], dt.float16, tag="zb")
            nc.scalar.dma_start(
                out=zbb0[:],
                in_=zr_d[0:8].unsqueeze(1).broadcast_to([8, 16, o_sh]))
            # stage DMAs for the first chunk of each startup tile go out
            # next (ACT ring); their casts are interleaved into the P0
            # dequant loop below so the first nibf casts aren't queued
            # behind them on the in-order ACT engine.
            x_stages = []
            for tt in range(FAST_X):
                st = xstage.tile([128, 8, T_TILE], dt.float32,
                                 tag="xs", name=f"xs_{tt}_0")
                nc.scalar.dma_start(out=st[:], in_=xt_d[tt, 0])
                x_stages.append(st)

            # ---- dequant phase: fp16 W tiles, ct = 8P + j
            # (startup x chunk waves interleaved between the P blocks)
            w_tiles = [None] * CT
            for P in range(NP_T):
                qw_t = qpool.tile([128, o_sh], dt.int32, tag="qw")
                nc.sync.dma_start(out=qw_t[:], in_=qw_d[P])
                if P == 0:
                    sbb, zbb = sbb0, zbb0  # pre-issued on the ACT ring
                else:
                    sbb = qpool.tile([128, o_sh], dt.float16, tag="sb")
                    nc.sync.dma_start(
                        out=sbb[:],
                        in_=sc_d[8 * P:8 * P + 8].unsqueeze(1)
                        .broadcast_to([8, 16, o_sh]))
                    zbb = qpool.tile([128, o_sh], dt.float16, tag="zb")
                    nc.sync.dma_start(
                        out=zbb[:],
                        in_=zr_d[8 * P:8 * P + 8].unsqueeze(1)
                        .broadcast_to([8, 16, o_sh]))
                for j in range(8):
                    ct = 8 * P + j
                    nib = wk.tile([128, o_sh], dt.int32, tag="nib")
                    nc.vector.tensor_scalar(
                        out=nib[:], in0=qw_t[:],
                        scalar1=4 * j, scalar2=15,
                        op0=mybir.AluOpType.logical_shift_right,
                        op1=mybir.AluOpType.bitwise_and)
                    # int32 -> fp16 on ScalarE: keeps the DVE TT ops in
                    # 16-bit 2x mode (bitVec TS can't cast on walrus)
                    nibf = wk.tile([128, o_sh], dt.float16, tag="nibf")
                    nc.scalar.copy(nibf[:], nib[:])
                    tmp = wk.tile([128, o_sh], dt.float16, tag="tmp")
                    nc.vector.tensor_tensor(
                        out=tmp[:], in0=nibf[:], in1=zbb[:],
                        op=mybir.AluOpType.subtract)
                    w = wpool.tile([128, o_sh], dt.float16, tag=f"w{ct}")
                    nc.vector.tensor_tensor(
                        out=w[:], in0=tmp[:], in1=sbb[:],
                        op=mybir.AluOpType.mult)
                    w_tiles[ct] = w
                    if P == 0 and j < FAST_X:
                        xr = xpool.tile([128, 8, T_TILE], dt.float16,
                                        tag=f"xr{P}", name=f"xr_{j}_0")
                        nc.scalar.copy(xr[:], x_stages[j][:])
                        xr_pre[j].append(xr)
                if P + 1 < NP_T:
                    for tt in range(FAST_X):
                        xr_pre[tt].append(emit_x_chunk(tt, P + 1))

            # ---- matmul phase
            o_tiles = _o_tiles(o_sh)

            def finish_group(tt, off, n, ps):
                t0 = tt * T_TILE
                ob = opool.tile([128, 512], dt.float32, tag="ob")
                nc.scalar.copy(ob[:, :n], ps[:, :n])
                nc.sync.dma_start(
                    out=y_d[t0:t0 + T_TILE, off:off + n], in_=ob[:, :n])

            # Early t-tiles run ct-outer across up to 8 psum groups (all
            # 8 banks) so each dequanted W tile feeds 8 matmuls as soon
            # as the DVE produces it (PE would otherwise idle behind the
            # dequant stream in program order).
            groups = [(tt, off, n) for tt in range(n_t)
                      for (off, n) in o_tiles]
            n_inter = min(8, len(groups)) if n_t > 1 else 0
            inter = [
                (tt, off, n,
                 pspool.tile([128, 512], dt.float32, tag="ps",
                             name=f"ps_i{tt}_{off}"))
                for tt, off, n in groups[:n_inter]]
            for ct in range(CT):
                for tt, off, n, ps in inter:
                    nc.tensor.matmul(
                        ps[:, :n],
                        lhsT=xr_pre[tt][ct // 8][:, ct % 8, :],
                        rhs=w_tiles[ct][:, off:off + n],
                        start=(ct == 0), stop=(ct == CT - 1))
            for tt, off, n, ps in inter:
                finish_group(tt, off, n, ps)

            last_tt = -1
            xr = None
            for tt, off, n in groups[n_inter:]:
                if tt != last_tt:
                    xr = xr_pre.get(tt) or emit_x(tt)
                    last_tt = tt
                ps = pspool.tile([128, 512], dt.float32, tag="ps")
                for ct in range(CT):
                    nc.tensor.matmul(
                        ps[:, :n],
                        lhsT=xr[ct // 8][:, ct % 8, :],
                        rhs=w_tiles[ct][:, off:off + n],
                        start=(ct == 0), stop=(ct == CT - 1))
                finish_group(tt, off, n, ps)

    nc.compile()
    return nc


_nc_cache = {}


def _get_nc(n_t=N_T, o_sh=O_SH):
    key = (n_t, o_sh)
    if key not in _nc_cache:
        _nc_cache[key] = _build(n_t, o_sh)
    return _nc_cache[key]


def _prep_inputs(x, qweight, qzeros, scales):
    """Host-side shard + layout prep (slicing/transposes only)."""
    x = np.asarray(x, dtype=np.float32)
    qweight = np.asarray(qweight, dtype=np.int32)
    qzeros = np.asarray(qzeros, dtype=np.int32)
    scales = np.asarray(scales, dtype=np.float32)

    t = x.shape[0]
    # xt[tt, P, pp, j, tl] = x[128*tt + tl, 1024P + 8pp + j]: each
    # SBUF partition line (j, tl) is contiguous in DRAM -> 4KB DMA
    # descriptor runs instead of 512B (3x DMA throughput on x loads).
    xk = np.ascontiguousarray(x.T)  # (K, t)
    xt = np.ascontiguousarray(
        xk.reshape(NP_T, 128, 8, t // T_TILE, T_TILE)
        .transpose(3, 0, 1, 2, 4))

    in_maps = []
    for c in range(N_CORES):
        sl = slice(c * O_SH, (c + 1) * O_SH)
        qsh = np.ascontiguousarray(qweight[sl].T).reshape(NP_T, 128, O_SH)
        in_maps.append({
            "xt": xt,
            "qw": qsh,
            "sc": scales[:, sl].astype(np.float16),
            "zr": qzeros[:, sl].astype(np.float16),
        })
    return in_maps


def run(x, qweight, qzeros, scales, trace=False, **trace_kwargs):
    """Full pipeline; returns (y, BassKernelResults)."""
    import time
    from concourse.bass_utils import run_bass_kernel_spmd

    nc = _get_nc()
    in_maps = _prep_inputs(x, qweight, qzeros, scales)
    last_err = None
    for attempt in range(3):
        try:
            res = run_bass_kernel_spmd(nc, in_maps, list(range(N_CORES)),
                                       trace=trace, **trace_kwargs)
            break
        except Exception as e:  # transient NRT device errors clear on retry
            last_err = e
            time.sleep(5 * (attempt + 1))
    else:
        raise last_err
    y = np.concatenate([r["y"] for r in res.results], axis=1)
    return y, res


def kernel(x, qweight, qzeros, scales):
    y, _ = run(x, qweight, qzeros, scales)
    return y


# revision 31
# speedup vs baseline: 1.0165x; 1.0165x over previous
"""AWQ int4 column-parallel linear for 8 Trainium2 NeuronCores.

y = x @ W^T, W[o,k] = (nib[o,k] - z[g,o]) * s[g,o], g = k // 128.

Sharding: out_features (11008) split into 8 contiguous shards of 1376
(column-parallel); x replicated; per-core outputs concatenated.

Per-core device kernel:
  - qweight shard arrives transposed+tiled (4, 128, 1376) int32
    [P, pp, o]; nibble j of pack (P, pp) is k = 1024*P + 8*pp + j.
  - dequant on DVE: (qw >> 4j) & 15 -> (nib - zb) * sb -> fp16 W tiles
    [128 k, 1376 o], one per contraction tile ct = 8P + j. zb/sb are
    replicated across the 16 partitions of each group via broadcast DMA.
  - x arrives pre-transposed/permuted (32, 128, 4096) fp32 [ct, pp, t];
    each token tile is DMA'd with an in-flight fp32->fp16 cast (SWDGE).
  - TensorE: 32 t-tiles x 3 o-tiles(512/512/352) x 32 k-tiles of fp16
    matmuls accumulated in PSUM fp32; ScalarE copies PSUM->SBUF; HWDGE
    stores to y.

fp16 keeps 10 mantissa bits on both operands (measured ~1.5e-4 rel err
class, vs 2.3e-3 for bf16) at bf16 throughput, and halves SBUF so the
full W shard stays resident and x streams exactly once.
"""

import numpy as np

TOKENS = 4096
IN_F = 4096
OUT_F = 11008
GROUP = 128
N_CORES = 8
O_SH = OUT_F // N_CORES      # 1376
PACKS = IN_F // 8            # 512
NP_T = PACKS // 128          # 4 pack-tiles
CT = NP_T * 8                # 32 contraction tiles
G = IN_F // GROUP            # 32 quant groups
T_TILE = 128
N_T = TOKENS // T_TILE       # 32 token tiles


def _o_tiles(o_sh):
    """Split the o shard into PSUM-sized (<=512) column tiles."""
    tiles = []
    off = 0
    while off < o_sh:
        n = min(512, o_sh - off)
        tiles.append((off, n))
        off += n
    return tiles


def _build(n_t=N_T, o_sh=O_SH):
    import concourse.bacc as bacc
    import concourse.mybir as mybir
    import concourse.tile as tile

    dt = mybir.dt
    nc = bacc.Bacc("TRN2", target_bir_lowering=False, debug=False)

    xt_d = nc.dram_tensor("xt", [n_t, NP_T, 128, 8, T_TILE], dt.float32,
                          kind="ExternalInput").ap()
    qw_d = nc.dram_tensor("qw", [NP_T, 128, o_sh], dt.int32,
                          kind="ExternalInput").ap()
    # scales/zeros arrive pre-broadcast from host: row p = group p//16
    sc_d = nc.dram_tensor("sc", [PACKS, o_sh], dt.float16,
                          kind="ExternalInput").ap()
    zr_d = nc.dram_tensor("zr", [PACKS, o_sh], dt.float16,
                          kind="ExternalInput").ap()
    y_d = nc.dram_tensor("y", [n_t * T_TILE, o_sh], dt.float32,
                         kind="ExternalOutput").ap()

    with tile.TileContext(nc) as tc:
        with (
            tc.tile_pool(name="qpool", bufs=2) as qpool,
            tc.tile_pool(name="wk", bufs=2) as wk,
            tc.tile_pool(name="wpool", bufs=1) as wpool,
            tc.tile_pool(name="xstage", bufs=4) as xstage,
            tc.tile_pool(name="xpool", bufs=4) as xpool,
            tc.tile_pool(name="opool", bufs=3) as opool,
            tc.tile_pool(name="psum", bufs=8, space="PSUM") as pspool,
        ):
            # x tile production: SWDGE casting DMA (fp32 DRAM -> fp16
            # SBUF), split into 4 chunk-DMAs per t-tile so the first
            # matmul gates on a 512KB chunk, not the whole 2MB (the
            # cast path runs well below line rate).
            def emit_x(tt):
                tiles = []
                for P in range(NP_T):
                    xr = xpool.tile([128, 8, T_TILE], dt.float16,
                                    tag=f"xr{P}", name=f"xr_{tt}_{P}")
                    nc.gpsimd.dma_start(out=xr[:], in_=xt_d[tt, P])
                    tiles.append(xr)
                return tiles
            # x tile production, startup: fp32 chunk DMA on the ACT
            # HWDGE ring (separate from the sync ring carrying qw/sc/
            # zr) + ACT cast. Chunks are emitted in waves matched to
            # the interleaved matmul block's progress through ct so
            # the DMA fabric isn't flooded ahead of the qw/zb/sb
            # transfers that gate dequant.
            def emit_x_chunk(tt, P):
                st = xstage.tile([128, 8, T_TILE], dt.float32,
                                 tag="xs", name=f"xs_{tt}_{P}")
                nc.scalar.dma_start(out=st[:], in_=xt_d[tt, P])
                xr = xpool.tile([128, 8, T_TILE], dt.float16,
                                tag=f"xr{P}", name=f"xr_{tt}_{P}")
                nc.scalar.copy(xr[:], st[:])
                return xr

            FAST_X = min(3, n_t)
            xr_pre = {tt: [] for tt in range(FAST_X)}
            # P0's scale/zero loads go FIRST on the ACT ring (FIFO per
            # ring — anything ahead of them delays the first W tile)
            sbb0 = qpool.tile([128, o_sh], dt.float16, tag="sb")
            nc.scalar.dma_start(out=sbb0[:], in_=sc_d[0:128])
            zbb0 = qpool.tile([128, o_sh], dt.float16, tag="zb")
            nc.scalar.dma_start(out=zbb0[:], in_=zr_d[0:128])
            # stage DMAs for the first chunk of each startup tile go out
            # next (ACT ring); their casts are interleaved into the P0
            # dequant loop below so the first nibf casts aren't queued
            # behind them on the in-order ACT engine.
            x_stages = []
            for tt in range(FAST_X):
                st = xstage.tile([128, 8, T_TILE], dt.float32,
                                 tag="xs", name=f"xs_{tt}_0")
                nc.scalar.dma_start(out=st[:], in_=xt_d[tt, 0])
                x_stages.append(st)

            # ---- dequant phase: fp16 W tiles, ct = 8P + j
            # (startup x chunk waves interleaved between the P blocks)
            w_tiles = [None] * CT
            for P in range(NP_T):
                qw_t = qpool.tile([128, o_sh], dt.int32, tag="qw")
                nc.sync.dma_start(out=qw_t[:], in_=qw_d[P])
                if P == 0:
                    sbb, zbb = sbb0, zbb0  # pre-issued on the ACT ring
                else:
                    sbb = qpool.tile([128, o_sh], dt.float16, tag="sb")
                    nc.sync.dma_start(
                        out=sbb[:], in_=sc_d[128 * P:128 * P + 128])
                    zbb = qpool.tile([128, o_sh], dt.float16, tag="zb")
                    nc.sync.dma_start(
                        out=zbb[:], in_=zr_d[128 * P:128 * P + 128])
                for j in range(8):
                    ct = 8 * P + j
                    nib = wk.tile([128, o_sh], dt.int32, tag="nib")
                    nc.vector.tensor_scalar(
                        out=nib[:], in0=qw_t[:],
                        scalar1=4 * j, scalar2=15,
                        op0=mybir.AluOpType.logical_shift_right,
                        op1=mybir.AluOpType.bitwise_and)
                    # int32 -> fp16 on ScalarE: keeps the DVE TT ops in
                    # 16-bit 2x mode (bitVec TS can't cast on walrus)
                    nibf = wk.tile([128, o_sh], dt.float16, tag="nibf")
                    nc.scalar.copy(nibf[:], nib[:])
                    tmp = wk.tile([128, o_sh], dt.float16, tag="tmp")
                    nc.vector.tensor_tensor(
                        out=tmp[:], in0=nibf[:], in1=zbb[:],
                        op=mybir.AluOpType.subtract)
                    w = wpool.tile([128, o_sh], dt.float16, tag=f"w{ct}")
                    nc.vector.tensor_tensor(
                        out=w[:], in0=tmp[:], in1=sbb[:],
                        op=mybir.AluOpType.mult)
                    w_tiles[ct] = w
                    if P == 0 and j < FAST_X:
                        xr = xpool.tile([128, 8, T_TILE], dt.float16,
                                        tag=f"xr{P}", name=f"xr_{j}_0")
                        nc.scalar.copy(xr[:], x_stages[j][:])
                        xr_pre[j].append(xr)
                if P + 1 < NP_T:
                    for tt in range(FAST_X):
                        xr_pre[tt].append(emit_x_chunk(tt, P + 1))

            # ---- matmul phase
            o_tiles = _o_tiles(o_sh)

            def finish_group(tt, off, n, ps):
                t0 = tt * T_TILE
                ob = opool.tile([128, 512], dt.float32, tag="ob")
                nc.scalar.copy(ob[:, :n], ps[:, :n])
                nc.sync.dma_start(
                    out=y_d[t0:t0 + T_TILE, off:off + n], in_=ob[:, :n])

            # Early t-tiles run ct-outer across up to 8 psum groups (all
            # 8 banks) so each dequanted W tile feeds 8 matmuls as soon
            # as the DVE produces it (PE would otherwise idle behind the
            # dequant stream in program order).
            groups = [(tt, off, n) for tt in range(n_t)
                      for (off, n) in o_tiles]
            n_inter = min(8, len(groups)) if n_t > 1 else 0
            inter = [
                (tt, off, n,
                 pspool.tile([128, 512], dt.float32, tag="ps",
                             name=f"ps_i{tt}_{off}"))
                for tt, off, n in groups[:n_inter]]
            for ct in range(CT):
                for tt, off, n, ps in inter:
                    nc.tensor.matmul(
                        ps[:, :n],
                        lhsT=xr_pre[tt][ct // 8][:, ct % 8, :],
                        rhs=w_tiles[ct][:, off:off + n],
                        start=(ct == 0), stop=(ct == CT - 1))
            for tt, off, n, ps in inter:
                finish_group(tt, off, n, ps)

            last_tt = -1
            xr = None
            for tt, off, n in groups[n_inter:]:
                if tt != last_tt:
                    xr = xr_pre.get(tt) or emit_x(tt)
                    last_tt = tt
                ps = pspool.tile([128, 512], dt.float32, tag="ps")
                for ct in range(CT):
                    nc.tensor.matmul(
                        ps[:, :n],
                        lhsT=xr[ct // 8][:, ct % 8, :],
                        rhs=w_tiles[ct][:, off:off + n],
                        start=(ct == 0), stop=(ct == CT - 1))
                finish_group(tt, off, n, ps)

    nc.compile()
    return nc


_nc_cache = {}


def _get_nc(n_t=N_T, o_sh=O_SH):
    key = (n_t, o_sh)
    if key not in _nc_cache:
        _nc_cache[key] = _build(n_t, o_sh)
    return _nc_cache[key]


def _prep_inputs(x, qweight, qzeros, scales):
    """Host-side shard + layout prep (slicing/transposes only)."""
    x = np.asarray(x, dtype=np.float32)
    qweight = np.asarray(qweight, dtype=np.int32)
    qzeros = np.asarray(qzeros, dtype=np.int32)
    scales = np.asarray(scales, dtype=np.float32)

    t = x.shape[0]
    # xt[tt, P, pp, j, tl] = x[128*tt + tl, 1024P + 8pp + j]: each
    # SBUF partition line (j, tl) is contiguous in DRAM -> 4KB DMA
    # descriptor runs instead of 512B (3x DMA throughput on x loads).
    xk = np.ascontiguousarray(x.T)  # (K, t)
    xt = np.ascontiguousarray(
        xk.reshape(NP_T, 128, 8, t // T_TILE, T_TILE)
        .transpose(3, 0, 1, 2, 4))

    in_maps = []
    for c in range(N_CORES):
        sl = slice(c * O_SH, (c + 1) * O_SH)
        qsh = np.ascontiguousarray(qweight[sl].T).reshape(NP_T, 128, O_SH)
        in_maps.append({
            "xt": xt,
            "qw": qsh,
            "sc": np.repeat(scales[:, sl].astype(np.float16), 16, axis=0),
            "zr": np.repeat(qzeros[:, sl].astype(np.float16), 16, axis=0),
        })
    return in_maps


def run(x, qweight, qzeros, scales, trace=False, **trace_kwargs):
    """Full pipeline; returns (y, BassKernelResults)."""
    import time
    from concourse.bass_utils import run_bass_kernel_spmd

    nc = _get_nc()
    in_maps = _prep_inputs(x, qweight, qzeros, scales)
    last_err = None
    for attempt in range(3):
        try:
            res = run_bass_kernel_spmd(nc, in_maps, list(range(N_CORES)),
                                       trace=trace, **trace_kwargs)
            break
        except Exception as e:  # transient NRT device errors clear on retry
            last_err = e
            time.sleep(5 * (attempt + 1))
    else:
        raise last_err
    y = np.concatenate([r["y"] for r in res.results], axis=1)
    return y, res


def kernel(x, qweight, qzeros, scales):
    y, _ = run(x, qweight, qzeros, scales)
    return y
